# revision 1
# baseline (speedup 1.0000x reference)
"""BlockwiseQuantLinear Trainium2 kernel.

y = x_deq @ w_deq.T where
  x_deq = fp8-blockwise-quantize-dequantize(x)  (1x128 blocks along K)
  w_deq = fp8 weight * 128x128 blockwise scales

Strategy: data-parallel over M across the 8 NeuronCores (8192 rows each).
Weight is dequantized to bf16 on host (small: 1024x1024), transposed to
[K, N], and replicated. On device, per 128-row m-tile:
  1. DVE: segmented abs-max over 1x128 blocks -> per-(row,block) scales.
     TRN2's fp8e4 is IEEE e4m3 (max 240), not e4m3fn (max 448), so we
     quantize with 224/amax: identical RTNE rounding up to a power of two.
  2. DVE: x * (224/amax) -> xq in fp8e4, and diag matrices diag(amax/224).
  3. PE: xq_block^T @ diag -> PSUM, which both transposes the block (K on
     partitions) and applies the dequant scale in one matmul.
  4. ACT: PSUM -> SBUF bf16 copies (x_deq^T tiles).
  5. PE: bf16 GEMM, 8 k-block matmuls accumulating into PSUM per 512-col
     half; ACT copies PSUM -> SBUF fp32; batched DMA out.
"""

import numpy as np
import ml_dtypes
from contextlib import ExitStack

import concourse.bass as bass
import concourse.bacc as bacc
import concourse.mybir as mybir
import concourse.tile as tile
from concourse.bass_utils import run_bass_kernel_spmd
from concourse.bass_interp import get_hw_module

M, K, N = 65536, 1024, 1024
NCORES = 8
MS = M // NCORES          # 8192 rows per core
B = 128                   # quant block size
KB = K // B               # 8 k-blocks
NB = N // B
GROUP = 4                 # m-tiles per DMA batch (2 MB transfers)
FP8_HW_MAX = 224.0        # trn2 fp8e4 is IEEE e4m3 (max 240); 224 = 448/2

F32 = mybir.dt.float32
BF16 = mybir.dt.bfloat16
FP8 = mybir.dt.float8e4


def build_bass(ms: int = MS, group: int = GROUP):
    """Build + compile the per-core Bass program for an ms-row shard."""
    mt = ms // B                      # m-tiles
    ngroups = mt // group
    assert mt % group == 0

    nc = bacc.Bacc(
        "TRN2", target_bir_lowering=False, debug=False, num_devices=NCORES
    )
    x_d = nc.dram_tensor("x", [ms, K], F32, kind="ExternalInput")
    wt_d = nc.dram_tensor("wt", [K, N], BF16, kind="ExternalInput")
    id_d = nc.dram_tensor("ident", [B, B], BF16, kind="ExternalInput")
    y_d = nc.dram_tensor("y", [ms, N], F32, kind="ExternalOutput")

    x_r = x_d.ap().rearrange("(g j p) k -> g p j k", p=B, j=group)
    y_r = y_d.ap().rearrange("(g j p) n -> g p j n", p=B, j=group)
    wt_r = wt_d.ap().rearrange("(kb p) n -> p kb n", p=B)

    with tile.TileContext(nc) as tc, ExitStack() as ctx:
        consts = ctx.enter_context(tc.tile_pool(name="consts", bufs=1))
        xin = ctx.enter_context(tc.tile_pool(name="xin", bufs=2))
        yout = ctx.enter_context(tc.tile_pool(name="yout", bufs=2))
        work = ctx.enter_context(tc.tile_pool(name="work", bufs=3))
        psum_t = ctx.enter_context(tc.tile_pool(name="psum_t", bufs=4, space="PSUM"))
        psum_y = ctx.enter_context(tc.tile_pool(name="psum_y", bufs=4, space="PSUM"))

        wt_s = consts.tile([B, KB, N], BF16)
        nc.sync.dma_start(wt_s[:], wt_r)
        ident = consts.tile([B, B], BF16)
        nc.sync.dma_start(ident[:], id_d.ap())

        for g in range(ngroups):
            xt = xin.tile([B, group, K], F32, tag="xt")
            nc.sync.dma_start(xt[:], x_r[g])
            yt = yout.tile([B, group, N], F32, tag="yt")

            for j in range(group):
                xmk = xt[:, j]                           # [128, 1024] f32

                amax = work.tile([B, KB], F32, tag="amax")
                nc.vector.tensor_reduce(
                    amax[:],
                    xmk.rearrange("p (kb b) -> p kb b", b=B),
                    axis=mybir.AxisListType.X,
                    op=mybir.AluOpType.max,
                    apply_absolute_value=True,
                )
                # clamp so xs = amax/448 >= 1e-12 as in the reference
                nc.vector.tensor_scalar_max(amax[:], amax[:], 448e-12)
                rxs = work.tile([B, KB], F32, tag="rxs")
                nc.vector.reciprocal(rxs[:], amax[:])
                nc.vector.tensor_scalar_mul(rxs[:], rxs[:], FP8_HW_MAX)
                xs = work.tile([B, KB], F32, tag="xs")
                nc.vector.tensor_scalar_mul(xs[:], amax[:], 1.0 / FP8_HW_MAX)

                # quantize: xq = fp8e4(x * 224/amax)
                xq = work.tile([B, K], FP8, tag="xq")
                diag8 = work.tile([B, KB, B], BF16, tag="diag8")
                for kb in range(KB):
                    nc.vector.tensor_scalar_mul(
                        xq[:, kb * B:(kb + 1) * B],
                        xmk[:, kb * B:(kb + 1) * B],
                        rxs[:, kb:kb + 1],
                    )
                    # diag(xs_kb) = I * xs (per-partition scalar)
                    nc.vector.tensor_scalar_mul(
                        diag8[:, kb], ident[:], xs[:, kb:kb + 1]
                    )

                # transpose + dequant: psum[k, m] = sum_m' xq[m',k] diag[m',m]
                pt0 = psum_t.tile([B, 512], F32, tag="pt")
                pt1 = psum_t.tile([B, 512], F32, tag="pt")
                for kb in range(KB):
                    pt = pt0 if kb < 4 else pt1
                    nc.tensor.matmul(
                        pt[:, (kb % 4) * B:(kb % 4 + 1) * B],
                        xq[:, kb * B:(kb + 1) * B],
                        diag8[:, kb],
                        start=True,
                        stop=True,
                    )
                xT = work.tile([B, K], BF16, tag="xT")
                nc.scalar.copy(xT[:, 0:512], pt0[:])
                nc.scalar.copy(xT[:, 512:1024], pt1[:])

                # main GEMM: y[m, :] = sum_kb xT_kb^T @ wT[kb]
                py0 = psum_y.tile([B, 512], F32, tag="py")
                py1 = psum_y.tile([B, 512], F32, tag="py")
                for kb in range(KB):
                    lhsT = xT[:, kb * B:(kb + 1) * B]
                    nc.tensor.matmul(
                        py0[:], lhsT, wt_s[:, kb, 0:512],
                        start=(kb == 0), stop=(kb == KB - 1),
                    )
                    nc.tensor.matmul(
                        py1[:], lhsT, wt_s[:, kb, 512:1024],
                        start=(kb == 0), stop=(kb == KB - 1),
                    )
                nc.scalar.copy(yt[:, j, 0:512], py0[:])
                nc.scalar.copy(yt[:, j, 512:1024], py1[:])

            nc.sync.dma_start(y_r[g], yt[:])

    nc.compile()
    nc.m = get_hw_module(nc.m)
    return nc


def host_prep(weight, w_scale):
    weight = np.asarray(weight)
    if weight.dtype != ml_dtypes.float8_e4m3fn:
        weight = weight.view(ml_dtypes.float8_e4m3fn)
    w_scale = np.asarray(w_scale, dtype=np.float32)
    nb, kb = w_scale.shape
    w_deq = (
        weight.astype(np.float32).reshape(nb, B, kb, B)
        * w_scale[:, None, :, None]
    ).reshape(nb * B, kb * B)
    wt = np.ascontiguousarray(w_deq.T).astype(ml_dtypes.bfloat16)  # [K, N]
    ident = np.eye(B, dtype=ml_dtypes.bfloat16)
    return wt, ident


_NC_CACHE = {}


def _get_nc(ms):
    if ms not in _NC_CACHE:
        _NC_CACHE[ms] = build_bass(ms)
    return _NC_CACHE[ms]


def kernel(x, weight, w_scale, _trace=False):
    x = np.ascontiguousarray(np.asarray(x, dtype=np.float32))
    assert x.shape == (M, K), x.shape
    wt, ident = host_prep(weight, w_scale)
    nc = _get_nc(MS)
    in_maps = [
        {"x": x[c * MS:(c + 1) * MS], "wt": wt, "ident": ident}
        for c in range(NCORES)
    ]
    res = run_bass_kernel_spmd(
        nc, in_maps, core_ids=list(range(NCORES)), trace=_trace
    )
    y = np.concatenate([r["y"] for r in res.results], axis=0)
    if _trace:
        return y, res
    return y
